# revision 1
# baseline (speedup 1.0000x reference)
"""Trainium2 Bass kernel for nn_MessagePassing (gnn_message_passing).

Math (per batch b):
    coef[s,e] = sum_o adj[s,o] * edge[s,o,e]
    v[s,e,i]  = sum_j W[e,i,j] * node[s,j]
    out[s,i]  = sum_e coef[s,e] * v[s,e,i]

Sharding: data parallel over the batch axis — core b handles batch b.
Per-core layout: s on SBUF partitions (tiles of 128 source nodes).
  * coef   -> DVE tensor_tensor_reduce per (s-tile, e): in0 = edge[s, o, e]
              (stride-E read over o), in1 = adj[s, o], add-reduce over o.
  * v      -> PE matmuls: lhsT = node^T[j, s-tile] (stationary),
              rhs = W[e]^T[j, i] (moving), out = psum[s, i].
  * out    -> chained scalar_tensor_tensor: acc = v_e * coef[:,e] + acc,
              with coef[:,e] as a per-partition scalar.
"""

import numpy as np
from contextlib import ExitStack

import concourse.bass as bass
import concourse.bacc as bacc
import concourse.mybir as mybir
import concourse.tile as tile
from concourse.bass_utils import run_bass_kernel_spmd
from concourse.masks import make_identity

B, N, D, E = 8, 1024, 128, 8
P = 128
NT = N // P  # 8 s-tiles per core

F32 = mybir.dt.float32
MUL = mybir.AluOpType.mult
ADD = mybir.AluOpType.add


def build_nc():
    nc = bacc.Bacc("TRN2", target_bir_lowering=False, debug=False, num_devices=B)

    node_d = nc.dram_tensor("node_state", [N, D], F32, kind="ExternalInput").ap()
    edge_d = nc.dram_tensor("edge_type_mat", [N, N, E], F32, kind="ExternalInput").ap()
    adj_d = nc.dram_tensor("adj_mat", [N, N], F32, kind="ExternalInput").ap()
    w_d = nc.dram_tensor("W", [E, D, D], F32, kind="ExternalInput").ap()
    out_d = nc.dram_tensor("out", [N, D], F32, kind="ExternalOutput").ap()

    with tile.TileContext(nc) as tc, ExitStack() as ctx:
        const_pool = ctx.enter_context(tc.tile_pool(name="const", bufs=1))
        edge_pool = ctx.enter_context(tc.tile_pool(name="edge", bufs=3))
        edge_r_pool = ctx.enter_context(tc.tile_pool(name="edge_r", bufs=2))
        work_pool = ctx.enter_context(tc.tile_pool(name="work", bufs=2))
        psum_pool = ctx.enter_context(tc.tile_pool(name="psum", bufs=8, space="PSUM"))

        ident = const_pool.tile([P, P], F32)
        make_identity(nc, ident[:])

        # Tile 0's edge stream goes FIRST into the Sync HWDGE FIFO (strict
        # FIFO per issuing engine) so the de-interleave + coef pipeline can
        # start as early as possible; setup loads queue behind it.
        edge_tiles = {}

        def load_edge(t):
            et = edge_pool.tile([P, N, E], F32, tag="edge_t")
            nc.sync.dma_start(et[:], edge_d[bass.ts(t, P)])
            return et

        edge_tiles[0] = load_edge(0)

        # Resident inputs. adj laid out [p, t, o] so slice t gives s-tile rows.
        adj_all = const_pool.tile([P, NT, N], F32)
        nc.sync.dma_start(adj_all[:], adj_d.rearrange("(t p) o -> p t o", p=P))
        node_all = const_pool.tile([P, NT, D], F32)
        nc.sync.dma_start(node_all[:], node_d.rearrange("(t p) j -> p t j", p=P))
        w_all = const_pool.tile([P, E, D], F32)  # [i, e, j]
        nc.sync.dma_start(w_all[:], w_d.rearrange("e i j -> i e j"))

        # node^T[j, s] and W[e]^T[j, i] via PE transpose.
        nodeT = const_pool.tile([P, N], F32)
        for t in range(NT):
            pt = psum_pool.tile([P, P], F32, tag="psum")
            nc.tensor.transpose(pt[:], node_all[:, t, :], ident[:])
            nc.scalar.copy(nodeT[:, bass.ts(t, P)], pt[:])
        wT = const_pool.tile([P, E, D], F32)  # [j, e, i]
        for e in range(E):
            pt = psum_pool.tile([P, P], F32, tag="psum")
            nc.tensor.transpose(pt[:], w_all[:, e, :], ident[:])
            nc.scalar.copy(wT[:, e, :], pt[:])

        scratch = const_pool.tile([P, N], F32)  # STT mandatory product output

        # e-slices 0..DEINT-1 are de-interleaved to contiguous [e][o] layout by
        # the (otherwise idle) ScalarE; the rest stay strided on VectorE.
        # (GpSimd is useless here: its SBUF port is exclusively shared with
        # VectorE, so GpSimd copies serialize against the DVE stream.)
        DEINT = 7

        for t in range(NT):
            edge_t = edge_tiles[t] if t in edge_tiles else load_edge(t)

            edge_r = edge_r_pool.tile([P, DEINT, N], F32, tag="edge_r")
            nc.scalar.copy(
                edge_r[:], edge_t[:, :, 0:DEINT].rearrange("p o e -> p e o")
            )

            coef = work_pool.tile([P, E], F32)
            for e in range(E):
                # coef[:, e] = sum_o edge[:, o, e] * adj[:, o]
                # (scalar_tensor_tensor: out = (in0 * 1.0) * in1, accum_out = sum(out))
                in0 = edge_r[:, e, :] if e < DEINT else edge_t[:, :, e]
                nc.vector.scalar_tensor_tensor(
                    out=scratch[:],
                    in0=in0,
                    scalar=1.0,
                    in1=adj_all[:, t, :],
                    op0=MUL,
                    op1=MUL,
                    accum_out=coef[:, e : e + 1],
                )

            # V[s, e, i] for 4 e's per matmul (N=512 moving operand).
            psums = []
            for g in range(E // 4):
                pv = psum_pool.tile([P, 4, D], F32, tag="psum")
                nc.tensor.matmul(
                    pv[:],
                    lhsT=nodeT[:, bass.ts(t, P)],
                    rhs=wT[:, g * 4 : (g + 1) * 4, :],
                    start=True,
                    stop=True,
                )
                psums.append(pv)

            acc_a = work_pool.tile([P, D], F32)
            acc_b = work_pool.tile([P, D], F32)
            nc.vector.tensor_scalar_mul(acc_a[:], psums[0][:, 0, :], coef[:, 0:1])
            cur, nxt = acc_a, acc_b
            for e in range(1, E):
                nc.vector.scalar_tensor_tensor(
                    out=nxt[:],
                    in0=psums[e // 4][:, e % 4, :],
                    scalar=coef[:, e : e + 1],
                    in1=cur[:],
                    op0=MUL,
                    op1=ADD,
                )
                cur, nxt = nxt, cur

            nc.gpsimd.dma_start(out_d[bass.ts(t, P)], cur[:])

    nc.compile()
    return nc


_NC_CACHE = None


def get_nc():
    global _NC_CACHE
    if _NC_CACHE is None:
        _NC_CACHE = build_nc()
    return _NC_CACHE


def make_in_maps(node_state, edge_type_mat, adj_mat, W):
    return [
        {
            "node_state": np.ascontiguousarray(node_state[b], dtype=np.float32),
            "edge_type_mat": np.ascontiguousarray(edge_type_mat[b], dtype=np.float32),
            "adj_mat": np.ascontiguousarray(adj_mat[b], dtype=np.float32),
            "W": np.ascontiguousarray(W, dtype=np.float32),
        }
        for b in range(B)
    ]


def kernel(node_state, edge_type_mat, adj_mat, W):
    nc = get_nc()
    in_maps = make_in_maps(node_state, edge_type_mat, adj_mat, W)
    res = run_bass_kernel_spmd(nc, in_maps, list(range(B)))
    return np.stack([res.results[b]["out"] for b in range(B)], axis=0)

